# revision 20
# baseline (speedup 1.0000x reference)
"""EvidenceNet pairwise-MLP scoring kernel for 8 Trainium2 NeuronCores.

Math (reference):
    img = sign(images_hash)/8, txt = sign(texts_hash)/8          [1024, 64] each
    a[i,k] = (img @ W1[:, :64].T)[i,k] + b1[k]                   [1024, 128]
    t[j,k] = (txt @ W1[:, 64:].T)[j,k]                           [1024, 128]
    negE[i,j] = sum_k W2[0,k] * relu(a[i,k] + t[j,k]) + b2[0]
    posE[i,j] = img[i,:] @ txt[j,:]
    out = [exp(clip(posE/0.5)), exp(clip(negE/0.5))] flattened   [1024*1024, 2]
    (clip at +-15 never binds: |2*negE| < 1, |2*posE| <= 2)

Distribution: data-parallel over image rows; core c owns i in [128c, 128c+128).

Host precomputes sign() (+-1 bf16-exact) and folds the 1/8 scales into W1,
b1 into an extra ones-row of the img operand, so the device starts matmuls
straight off the DMA.

Per-core device program (k = the 128 hidden dims lives on partitions):
    aT   [128k, 128i]   = w65[:, :128]^T-matmul of imgT65 (b1 folded)  (f32)
    tT_h [128k, 1024j]  = w65[:64, 128:]^T-matmul of txtT  (bf16, 2 chunks:
                          chunk0 cast on VectorE, chunk1 on ScalarE)
    per i (~95 rows on VectorE at 4x bf16, ~33 rows on ScalarE):
        r_i [128k, 1024j] = relu(tT_h + aT[:, i])           (bf16)
        for jb in 0..8:  # contiguous lhsT, negE lands transposed
            psum[jb//4][:, (jb%4)*128+i] = matmul(lhsT=r_i[:, jb*128:+128],
                                                  rhs=w2c)
    negT2 = exp(2*psum + 2*b2) in 3 i-phases, i-major cols   -> [128jr, 8*128]
    out_pos = exp(posE/32), posE = sign-img x sign-txt matmul (exact bf16)
Host gathers: col0 = pos rows, col1 from negO (negO[jr, i*8+jb] =
negE[i, jb*128+jr]), concat.
"""
import numpy as np
import ml_dtypes

N_CORES = 8
NI, NT, D, H = 1024, 1024, 64, 128
NI_LOC = NI // N_CORES  # 128
NJB = NT // H           # 8 psum column-blocks of 128 j
R_BUFS = 40             # in-flight relu tiles (DVE/ACT run-ahead over PE)

_compiled = None


ACT_NUM, ACT_DEN = 33, 128  # fraction of relu rows on ScalarE
SPLIT_ROWS = 6           # first rows emit half-width relu (start before full tT_h)


def _engine_map():
    """Per-i relu engine: 'A' (ScalarE), 'V' (VectorE).

    The last TAIL_V rows are forced to VectorE (A rows are 2.6x longer, so an
    A row near the end extends the final eviction's critical path)."""
    TAIL_V = 8
    eng = []
    acc = 0
    for i in range(NI_LOC):
        acc += ACT_NUM
        if acc >= ACT_DEN:
            acc -= ACT_DEN
            eng.append("A")
        else:
            eng.append("V")
    for i in range(NI_LOC - TAIL_V, NI_LOC):
        if eng[i] == "A":
            eng[i] = "V"
            # re-home the displaced A row just before the tail window
            for j in range(NI_LOC - TAIL_V - 1, 0, -1):
                if eng[j] == "V":
                    eng[j] = "A"
                    break
    return eng


def _build():
    import concourse.bacc as bacc
    import concourse.tile as tile
    import concourse.mybir as mybir

    F32 = mybir.dt.float32
    BF16 = mybir.dt.bfloat16
    AF = mybir.ActivationFunctionType
    ALU = mybir.AluOpType

    nc = bacc.Bacc("TRN2", target_bir_lowering=False, debug=False,
                   num_devices=N_CORES)

    # small critical input [imgT65 | w65] (feeds aT) + the big txtT separately,
    # so aT's matmul isn't gated on the 128KB txtT transfer
    SW = NI_LOC + 2 * H
    sm_d = nc.dram_tensor("sm", [D + 1, SW], BF16, kind="ExternalInput").ap()
    txtT_d = nc.dram_tensor("txtT", [D, NT], BF16, kind="ExternalInput").ap()
    w2c_d = nc.dram_tensor("w2c", [H, 1], BF16, kind="ExternalInput").ap()
    b2s_d = nc.dram_tensor("b2s", [H, 1], F32, kind="ExternalInput").ap()
    pos_d = nc.dram_tensor("pos", [NI_LOC, NT], F32, kind="ExternalOutput").ap()
    # negO i-major: negO[jr, i*8 + jb] = negE[i, jb*128 + jr]
    negO_d = nc.dram_tensor("negO", [H, NT], F32, kind="ExternalOutput").ap()

    eng_map = _engine_map()
    CH = 512  # setup pipeline chunk

    with tile.TileContext(nc) as tc:
        with tc.tile_pool(name="const", bufs=1) as cpool, \
             tc.tile_pool(name="rp", bufs=R_BUFS) as rpool, \
             tc.tile_pool(name="op", bufs=1) as opool:

            # ---- load inputs (host already applied sign/transposes) ----------
            # Single packed DMA triggered from the GpSimd queue (reaches its
            # first instruction ~1us before the Sync queue); all matmul inputs
            # land together so the scheduler keeps the aT matmul first.
            sm65 = cpool.tile([D + 1, SW], BF16)
            nc.gpsimd.dma_start(sm65[:], sm_d[:])
            txtT_s = cpool.tile([D, NT], BF16)
            nc.gpsimd.dma_start(txtT_s[:], txtT_d[:])
            W0 = NI_LOC  # w65 column offset inside sm65
            # non-critical (first used at ~13us / ~35us): Sync queue is fine
            w2c = cpool.tile([H, 1], BF16)
            nc.sync.dma_start(w2c[:], w2c_d[:])
            b2s = cpool.tile([H, 1], F32)
            nc.sync.dma_start(b2s[:], b2s_d[:])

            # ---- trigger the ACT table load early (no input deps) ------------
            warm = cpool.tile([1, 1], F32)
            nc.vector.memset(warm[:], 0.0)
            nc.scalar.activation(warm[:], warm[:], AF.Exp, bias=0.0, scale=1.0)

            tT_h = cpool.tile([H, NT], BF16)
            aT = cpool.tile([H, NI_LOC], F32)
            pos_sb = opool.tile([NI_LOC, NT], F32)

            with tc.tile_pool(name="ps_set", bufs=2, space="PSUM") as ps_s, \
                 tc.tile_pool(name="ps_a", bufs=1, space="PSUM") as ps_a:
                # aT with b1 folded via the ones-row of imgT65
                aps = ps_a.tile([H, NI_LOC], F32)
                nc.tensor.matmul(aps[:], lhsT=sm65[:, W0:W0 + H],
                                 rhs=sm65[:, 0:NI_LOC],
                                 start=True, stop=True)
                nc.vector.tensor_copy(aT[:], aps[:])

                # tT_h: chunk 0 cast on VectorE (feeds first relu rows),
                # chunk 1 on ScalarE (lands while V streams half-width rows)
                for ci, hh in enumerate(range(0, NT, CH)):
                    ps = ps_s.tile([H, CH], F32, tag="hps")
                    nc.tensor.matmul(ps[:],
                                     lhsT=sm65[0:D, W0 + H:W0 + 2 * H],
                                     rhs=txtT_s[:, hh:hh + CH],
                                     start=True, stop=True)
                    if ci == 0:
                        nc.vector.tensor_copy(tT_h[:, hh:hh + CH], ps[:])
                    else:
                        nc.scalar.activation(tT_h[:, hh:hh + CH], ps[:],
                                             AF.Copy, bias=0.0, scale=1.0)

                for hh in range(0, NT, 512):
                    ps = ps_s.tile([NI_LOC, 512], F32, tag="pps")
                    nc.tensor.matmul(ps[:], lhsT=sm65[0:D, 0:NI_LOC],
                                     rhs=txtT_s[:, hh:hh + 512],
                                     start=True, stop=True)
                    nc.scalar.activation(pos_sb[:, hh:hh + 512], ps[:],
                                         AF.Exp, bias=0.0, scale=1.0 / 32.0)
            nc.sync.dma_start(pos_d[:], pos_sb[:])

            # ---- main pairwise loop (negE transposed: psum pair p holds
            #      jb=4p..4p+3 as [128j, 4*128i])
            with tc.tile_pool(name="ps_m", bufs=1, space="PSUM") as ps_m:
                # 4 jb-blocks per tile: [128, 512] f32 = exactly one PSUM bank
                psums = [ps_m.tile([H, 4 * NI_LOC], F32, tag=f"np{p}",
                                   name=f"negps{p}")
                         for p in range(NJB // 4)]
                negT2 = opool.tile([H, NT], F32)
                phases = [(0, 64), (64, 96), (96, 120), (120, NI_LOC)]
                HW_ = NT // 2

                def emit_evict(i0, i1):
                    for p in range(NJB // 4):
                        # negT2[j, i*8 + 4p+s] <- exp(2*psum[j, s*128+i] + 2*b2)
                        nc.scalar.activation(
                            negT2[:, :].rearrange(
                                "j (i b) -> j b i",
                                b=NJB)[:, 4 * p:4 * p + 4, i0:i1],
                            psums[p][:, :].rearrange(
                                "j (s i) -> j s i", s=4)[:, :, i0:i1],
                            AF.Exp, bias=b2s[:], scale=2.0)
                    nc.sync.dma_start(
                        negO_d[:, i0 * NJB:i1 * NJB],
                        negT2[:, i0 * NJB:i1 * NJB])

                pending = None
                for i0, i1 in phases:
                    # defer the previous phase's eviction a few rows in so it
                    # never head-of-line-blocks ScalarE's relu stream (clamped
                    # so short phases still emit it)
                    emit_at = i0 + min(8, i1 - i0 - 1)
                    for i in range(i0, i1):
                        if pending is not None and i == emit_at:
                            emit_evict(*pending)
                            pending = None
                        if i < SPLIT_ROWS:
                            # two tiles so jb<4 matmuls only wait the lo half
                            r_lo = rpool.tile([H, HW_], BF16, tag="rlo")
                            r_hi = rpool.tile([H, HW_], BF16, tag="rhi")
                            parts = [(r_lo, 0), (r_hi, HW_)]
                        else:
                            r = rpool.tile([H, NT], BF16, tag="r")
                            parts = [(r, 0)]
                        for rt, off in parts:
                            w = HW_ if i < SPLIT_ROWS else NT
                            if eng_map[i] == "A":
                                nc.scalar.activation(rt[:],
                                                     tT_h[:, off:off + w],
                                                     AF.Relu,
                                                     bias=aT[:, i:i + 1],
                                                     scale=1.0)
                            else:
                                nc.vector.tensor_scalar(rt[:],
                                                        tT_h[:, off:off + w],
                                                        aT[:, i:i + 1], 0.0,
                                                        op0=ALU.add,
                                                        op1=ALU.max)
                        for jb in range(NJB):
                            col = (jb % 4) * NI_LOC + i
                            if i < SPLIT_ROWS:
                                rt = parts[jb // 4][0]
                                lhsT = rt[:, (jb % 4) * H:(jb % 4 + 1) * H]
                            else:
                                lhsT = parts[0][0][:, jb * H:(jb + 1) * H]
                            nc.tensor.matmul(psums[jb // 4][:, col:col + 1],
                                             lhsT=lhsT,
                                             rhs=w2c[:], start=True, stop=True)
                    pending = (i0, i1)
                emit_evict(*pending)

    nc.compile()
    return nc


def _get_compiled():
    global _compiled
    if _compiled is None:
        _compiled = _build()
    return _compiled


def run(inputs: dict, trace: bool = False):
    """Shard, run on 8 cores, gather. Returns (full_output, BassKernelResults)."""
    from concourse.bass_utils import run_bass_kernel_spmd

    nc = _get_compiled()

    imgs = np.asarray(inputs["images_hash"], dtype=np.float32)
    txts = np.asarray(inputs["texts_hash"], dtype=np.float32)
    W1 = np.asarray(inputs["W1"], dtype=np.float32)
    b1 = np.asarray(inputs["b1"], dtype=np.float32)
    W2 = np.asarray(inputs["W2"], dtype=np.float32)
    b2 = np.asarray(inputs["b2"], dtype=np.float32)
    task = int(np.asarray(inputs["task_is_i2t"]))

    bf16 = ml_dtypes.bfloat16
    # sign() on host: +-1 exact in bf16; 1/8 scales folded into W1 cols
    # packed input big65 [65, 1408] = [txtT | imgT65 | w65]
    w65 = np.concatenate([
        np.concatenate([W1[:, :D].T * 0.125, b1[None, :]], axis=0),
        np.concatenate([W1[:, D:].T * 0.125, np.zeros((1, H), np.float32)],
                       axis=0)], axis=1)                            # [65, 256]
    txtT65 = np.concatenate(
        [np.sign(txts).T, np.zeros((1, NT), np.float32)], axis=0)   # [65, 1024]
    w2c = W2[0][:, None].astype(bf16)                               # [128, 1]
    b2s = np.full((H, 1), 2.0 * float(b2[0]), np.float32)

    txtT = txtT65[0:D].astype(bf16)
    in_maps = []
    for c in range(N_CORES):
        sl = imgs[c * NI_LOC:(c + 1) * NI_LOC]
        imgT65 = np.concatenate(
            [np.sign(sl).T, np.ones((1, NI_LOC), np.float32)], axis=0)
        sm = np.concatenate([imgT65, w65], axis=1).astype(bf16)
        in_maps.append({"sm": sm, "txtT": txtT, "w2c": w2c, "b2s": b2s})

    res = run_bass_kernel_spmd(nc, in_maps, list(range(N_CORES)), trace=trace)

    full = np.empty((NI * NT, 2), dtype=np.float32)
    pos = np.concatenate([res.results[c]["pos"] for c in range(N_CORES)], axis=0)
    # negO[jr, i*8+jb] = negE[i, jb*128+jr]  ->  neg_core[i, j]
    neg = np.concatenate(
        [res.results[c]["negO"].reshape(H, NI_LOC, NJB).transpose(1, 2, 0)
         .reshape(NI_LOC, NT) for c in range(N_CORES)], axis=0)
    full[:, 0] = (pos if task else pos.T).reshape(-1)
    full[:, 1] = neg.reshape(-1)
    return full, res


def kernel(**inputs) -> np.ndarray:
    out, _ = run(inputs, trace=False)
    return out


# revision 22
# speedup vs baseline: 1.0505x; 1.0505x over previous
"""EvidenceNet pairwise-MLP scoring kernel for 8 Trainium2 NeuronCores.

Math (reference):
    img = sign(images_hash)/8, txt = sign(texts_hash)/8          [1024, 64] each
    a[i,k] = (img @ W1[:, :64].T)[i,k] + b1[k]                   [1024, 128]
    t[j,k] = (txt @ W1[:, 64:].T)[j,k]                           [1024, 128]
    negE[i,j] = sum_k W2[0,k] * relu(a[i,k] + t[j,k]) + b2[0]
    posE[i,j] = img[i,:] @ txt[j,:]
    out = [exp(clip(posE/0.5)), exp(clip(negE/0.5))] flattened   [1024*1024, 2]
    (clip at +-15 never binds: |2*negE| < 1, |2*posE| <= 2)

Distribution: data-parallel over image rows; core c owns i in [128c, 128c+128).

Host precomputes sign() (+-1 bf16-exact) and folds the 1/8 scales into W1,
b1 into an extra ones-row of the img operand, so the device starts matmuls
straight off the DMA.

Per-core device program (k = the 128 hidden dims lives on partitions):
    aT   [128k, 128i]   = w65[:, :128]^T-matmul of imgT65 (b1 folded)  (f32)
    tT_h [128k, 1024j]  = w65[:64, 128:]^T-matmul of txtT  (bf16, 2 chunks:
                          chunk0 cast on VectorE, chunk1 on ScalarE)
    per i (~95 rows on VectorE at 4x bf16, ~33 rows on ScalarE):
        r_i [128k, 1024j] = relu(tT_h + aT[:, i])           (bf16)
        for jb in 0..8:  # contiguous lhsT, negE lands transposed
            psum[jb//4][:, (jb%4)*128+i] = matmul(lhsT=r_i[:, jb*128:+128],
                                                  rhs=w2c)
    negT2 = exp(2*psum + 2*b2) in 3 i-phases, i-major cols   -> [128jr, 8*128]
    out_pos = exp(posE/32), posE = sign-img x sign-txt matmul (exact bf16)
Host gathers: col0 = pos rows, col1 from negO (negO[jr, i*8+jb] =
negE[i, jb*128+jr]), concat.
"""
import numpy as np
import ml_dtypes

N_CORES = 8
NI, NT, D, H = 1024, 1024, 64, 128
NI_LOC = NI // N_CORES  # 128
NJB = NT // H           # 8 psum column-blocks of 128 j
R_BUFS = 40             # in-flight relu tiles (DVE/ACT run-ahead over PE)

_compiled = None


ACT_NUM, ACT_DEN = 33, 128  # fraction of relu rows on ScalarE
SPLIT_ROWS = 6           # first rows emit half-width relu (start before full tT_h)


def _engine_map():
    """Per-i relu engine: 'A' (ScalarE), 'V' (VectorE).

    The last TAIL_V rows are forced to VectorE (A rows are 2.6x longer, so an
    A row near the end extends the final eviction's critical path)."""
    TAIL_V = 8
    eng = []
    acc = 0
    for i in range(NI_LOC):
        acc += ACT_NUM
        if acc >= ACT_DEN:
            acc -= ACT_DEN
            eng.append("A")
        else:
            eng.append("V")
    for i in range(NI_LOC - TAIL_V, NI_LOC):
        if eng[i] == "A":
            eng[i] = "V"
            # re-home the displaced A row just before the tail window
            for j in range(NI_LOC - TAIL_V - 1, 0, -1):
                if eng[j] == "V":
                    eng[j] = "A"
                    break
    return eng


def _build():
    import concourse.bacc as bacc
    import concourse.tile as tile
    import concourse.mybir as mybir

    F32 = mybir.dt.float32
    BF16 = mybir.dt.bfloat16
    AF = mybir.ActivationFunctionType
    ALU = mybir.AluOpType

    nc = bacc.Bacc("TRN2", target_bir_lowering=False, debug=False,
                   num_devices=N_CORES)

    # small critical input [imgT65 | w65] (feeds aT) + the big txtT separately,
    # so aT's matmul isn't gated on the 128KB txtT transfer
    SW = NI_LOC + 2 * H
    sm_d = nc.dram_tensor("sm", [D + 1, SW], BF16, kind="ExternalInput").ap()
    txtT_d = nc.dram_tensor("txtT", [D, NT], BF16, kind="ExternalInput").ap()
    w2c_d = nc.dram_tensor("w2c", [H, 1], BF16, kind="ExternalInput").ap()
    b2s_d = nc.dram_tensor("b2s", [H, 1], F32, kind="ExternalInput").ap()
    pos_d = nc.dram_tensor("pos", [NI_LOC, NT], F32, kind="ExternalOutput").ap()
    # negO i-major: negO[jr, i*8 + jb] = negE[i, jb*128 + jr]
    negO_d = nc.dram_tensor("negO", [H, NT], F32, kind="ExternalOutput").ap()

    eng_map = _engine_map()
    CH = 512  # setup pipeline chunk

    with tile.TileContext(nc) as tc:
        with tc.tile_pool(name="const", bufs=1) as cpool, \
             tc.tile_pool(name="rp", bufs=R_BUFS) as rpool, \
             tc.tile_pool(name="op", bufs=1) as opool:

            # ---- load inputs (host already applied sign/transposes) ----------
            # Single packed DMA triggered from the GpSimd queue (reaches its
            # first instruction ~1us before the Sync queue); all matmul inputs
            # land together so the scheduler keeps the aT matmul first.
            # HWDGE paths (scalar/sync) have ~0.6us first-byte latency vs
            # ~1us + slow issue on the gpsimd SWDGE path
            sm65 = cpool.tile([D + 1, SW], BF16)
            nc.scalar.dma_start(sm65[:], sm_d[:])
            txtT_s = cpool.tile([D, NT], BF16)
            nc.sync.dma_start(txtT_s[:], txtT_d[:])
            W0 = NI_LOC  # w65 column offset inside sm65
            # non-critical (first used at ~13us / ~35us)
            w2c = cpool.tile([H, 1], BF16)
            nc.sync.dma_start(w2c[:], w2c_d[:])
            b2s = cpool.tile([H, 1], F32)
            nc.sync.dma_start(b2s[:], b2s_d[:])

            # ---- trigger the ACT table load early (no input deps) ------------
            warm = cpool.tile([1, 1], F32)
            nc.vector.memset(warm[:], 0.0)
            nc.scalar.activation(warm[:], warm[:], AF.Exp, bias=0.0, scale=1.0)

            tT_h = cpool.tile([H, NT], BF16)
            aT = cpool.tile([H, NI_LOC], F32)
            pos_sb = opool.tile([NI_LOC, NT], F32)

            with tc.tile_pool(name="ps_set", bufs=2, space="PSUM") as ps_s, \
                 tc.tile_pool(name="ps_a", bufs=1, space="PSUM") as ps_a:
                # aT with b1 folded via the ones-row of imgT65
                aps = ps_a.tile([H, NI_LOC], F32)
                nc.tensor.matmul(aps[:], lhsT=sm65[:, W0:W0 + H],
                                 rhs=sm65[:, 0:NI_LOC],
                                 start=True, stop=True)
                nc.vector.tensor_copy(aT[:], aps[:])

                # tT_h: chunk 0 cast on VectorE (feeds first relu rows),
                # chunk 1 on ScalarE (lands while V streams half-width rows)
                for ci, hh in enumerate(range(0, NT, CH)):
                    ps = ps_s.tile([H, CH], F32, tag="hps")
                    nc.tensor.matmul(ps[:],
                                     lhsT=sm65[0:D, W0 + H:W0 + 2 * H],
                                     rhs=txtT_s[:, hh:hh + CH],
                                     start=True, stop=True)
                    if ci == 0:
                        nc.vector.tensor_copy(tT_h[:, hh:hh + CH], ps[:])
                    else:
                        nc.scalar.activation(tT_h[:, hh:hh + CH], ps[:],
                                             AF.Copy, bias=0.0, scale=1.0)

                for hh in range(0, NT, 512):
                    ps = ps_s.tile([NI_LOC, 512], F32, tag="pps")
                    nc.tensor.matmul(ps[:], lhsT=sm65[0:D, 0:NI_LOC],
                                     rhs=txtT_s[:, hh:hh + 512],
                                     start=True, stop=True)
                    nc.scalar.activation(pos_sb[:, hh:hh + 512], ps[:],
                                         AF.Exp, bias=0.0, scale=1.0 / 32.0)
            nc.sync.dma_start(pos_d[:], pos_sb[:])

            # ---- main pairwise loop (negE transposed: per-phase psum pair p
            #      holds jb=4p..4p+3 as [128j, 4*width]; per-phase tiles so an
            #      eviction read never WAR-blocks the next phase's matmuls)
            with tc.tile_pool(name="ps_m", bufs=1, space="PSUM") as ps_m:
                negT2 = opool.tile([H, NT], F32)
                phases = [(0, 64), (64, 96), (96, 120), (120, NI_LOC)]
                HW_ = NT // 2
                psums_ph = {
                    (ph, p): ps_m.tile([H, 4 * (i1 - i0)], F32,
                                       tag=f"np{ph}_{p}", name=f"negps{ph}_{p}")
                    for ph, (i0, i1) in enumerate(phases)
                    for p in range(NJB // 4)}

                def emit_evict(ph):
                    i0, i1 = phases[ph]
                    for p in range(NJB // 4):
                        # negT2[j, i*8 + 4p+s] <- exp(2*psum[j, s*w+i-i0] + 2*b2)
                        nc.scalar.activation(
                            negT2[:, :].rearrange(
                                "j (i b) -> j b i",
                                b=NJB)[:, 4 * p:4 * p + 4, i0:i1],
                            psums_ph[ph, p][:, :].rearrange(
                                "j (s i) -> j s i", s=4),
                            AF.Exp, bias=b2s[:], scale=2.0)
                    nc.sync.dma_start(
                        negO_d[:, i0 * NJB:i1 * NJB],
                        negT2[:, i0 * NJB:i1 * NJB])

                pending = None
                for ph, (i0, i1) in enumerate(phases):
                    # defer the previous phase's eviction a few rows in so it
                    # lands on the ScalarE queue where relu progress has
                    # already covered the awaited matmuls (clamped for the
                    # short final phase)
                    emit_at = i0 + min(8, i1 - i0 - 1)
                    wph = i1 - i0
                    for i in range(i0, i1):
                        if pending is not None and i == emit_at:
                            emit_evict(pending)
                            pending = None
                        if i < SPLIT_ROWS:
                            # two tiles so jb<4 matmuls only wait the lo half
                            r_lo = rpool.tile([H, HW_], BF16, tag="rlo")
                            r_hi = rpool.tile([H, HW_], BF16, tag="rhi")
                            parts = [(r_lo, 0), (r_hi, HW_)]
                        else:
                            r = rpool.tile([H, NT], BF16, tag="r")
                            parts = [(r, 0)]
                        for rt, off in parts:
                            w = HW_ if i < SPLIT_ROWS else NT
                            if eng_map[i] == "A":
                                nc.scalar.activation(rt[:],
                                                     tT_h[:, off:off + w],
                                                     AF.Relu,
                                                     bias=aT[:, i:i + 1],
                                                     scale=1.0)
                            else:
                                nc.vector.tensor_scalar(rt[:],
                                                        tT_h[:, off:off + w],
                                                        aT[:, i:i + 1], 0.0,
                                                        op0=ALU.add,
                                                        op1=ALU.max)
                        for jb in range(NJB):
                            col = (jb % 4) * wph + (i - i0)
                            if i < SPLIT_ROWS:
                                rt = parts[jb // 4][0]
                                lhsT = rt[:, (jb % 4) * H:(jb % 4 + 1) * H]
                            else:
                                lhsT = parts[0][0][:, jb * H:(jb + 1) * H]
                            nc.tensor.matmul(
                                psums_ph[ph, jb // 4][:, col:col + 1],
                                lhsT=lhsT,
                                rhs=w2c[:], start=True, stop=True)
                    pending = ph
                emit_evict(pending)

    nc.compile()
    return nc


def _get_compiled():
    global _compiled
    if _compiled is None:
        _compiled = _build()
    return _compiled


def run(inputs: dict, trace: bool = False):
    """Shard, run on 8 cores, gather. Returns (full_output, BassKernelResults)."""
    from concourse.bass_utils import run_bass_kernel_spmd

    nc = _get_compiled()

    imgs = np.asarray(inputs["images_hash"], dtype=np.float32)
    txts = np.asarray(inputs["texts_hash"], dtype=np.float32)
    W1 = np.asarray(inputs["W1"], dtype=np.float32)
    b1 = np.asarray(inputs["b1"], dtype=np.float32)
    W2 = np.asarray(inputs["W2"], dtype=np.float32)
    b2 = np.asarray(inputs["b2"], dtype=np.float32)
    task = int(np.asarray(inputs["task_is_i2t"]))

    bf16 = ml_dtypes.bfloat16
    # sign() on host: +-1 exact in bf16; 1/8 scales folded into W1 cols
    # packed input big65 [65, 1408] = [txtT | imgT65 | w65]
    w65 = np.concatenate([
        np.concatenate([W1[:, :D].T * 0.125, b1[None, :]], axis=0),
        np.concatenate([W1[:, D:].T * 0.125, np.zeros((1, H), np.float32)],
                       axis=0)], axis=1)                            # [65, 256]
    txtT65 = np.concatenate(
        [np.sign(txts).T, np.zeros((1, NT), np.float32)], axis=0)   # [65, 1024]
    w2c = W2[0][:, None].astype(bf16)                               # [128, 1]
    b2s = np.full((H, 1), 2.0 * float(b2[0]), np.float32)

    txtT = txtT65[0:D].astype(bf16)
    in_maps = []
    for c in range(N_CORES):
        sl = imgs[c * NI_LOC:(c + 1) * NI_LOC]
        imgT65 = np.concatenate(
            [np.sign(sl).T, np.ones((1, NI_LOC), np.float32)], axis=0)
        sm = np.concatenate([imgT65, w65], axis=1).astype(bf16)
        in_maps.append({"sm": sm, "txtT": txtT, "w2c": w2c, "b2s": b2s})

    res = run_bass_kernel_spmd(nc, in_maps, list(range(N_CORES)), trace=trace)

    full = np.empty((NI * NT, 2), dtype=np.float32)
    pos = np.concatenate([res.results[c]["pos"] for c in range(N_CORES)], axis=0)
    # negO[jr, i*8+jb] = negE[i, jb*128+jr]  ->  neg_core[i, j]
    neg = np.concatenate(
        [res.results[c]["negO"].reshape(H, NI_LOC, NJB).transpose(1, 2, 0)
         .reshape(NI_LOC, NT) for c in range(N_CORES)], axis=0)
    full[:, 0] = (pos if task else pos.T).reshape(-1)
    full[:, 1] = neg.reshape(-1)
    return full, res


def kernel(**inputs) -> np.ndarray:
    out, _ = run(inputs, trace=False)
    return out
